# revision 14
# baseline (speedup 1.0000x reference)
"""Multi-head attention TRN2 Bass kernel, head-sharded across 8 NeuronCores.

Problem: S=2048, E=1024, H=16 heads, dk=dv=64, fp32.
    Q = x @ Wq.T ; K = x @ Wk.T ; V = x @ Wv.T   (per-head slices)
    A_h = softmax(Q_h K_h^T / 8) V_h
    out = concat_h(A_h) @ Wo.T
Sharding: tensor-parallel over heads. Core i owns heads (2i, 2i+1); the 8
partial [2048,1024] outputs are summed on the host.

v6 layout (per-core):
  * Few, large HWDGE input transfers; first x quarter split across both
    queues so the first projection starts earliest. All y output DMAs ride
    the sync queue only -- a dma_start costs ~0.6us of issuing-engine time
    and the scalar engine (exp) is the kernel bottleneck.
  * 16 warm-up matmuls at t~0 open the HAM clock gate before real work.
  * Per-quarter pipeline: K_t -> scores granule -> V_t^T -> PE transposes ->
    scores granule -> Q_t, with block-0 attention interleaved so exp starts
    as soon as quarter 0 lands.
  * finish_block(b) (normalize + output projection) is emitted in the middle
    of block b+1 so its PE work fills the scores/AV pipeline instead of
    draining ACT at block boundaries.
  * PSUM: scores 2x[128,1024] + AV accum 2x[128,512] + kq 1 + v 1 = 8 banks.
  * y written per 128-row chunk as one [128,1024] DMA into a [16,128,E]
    DRAM layout (host reshape is free).
All matmul operands bf16 (fp32 PSUM accumulation). AV rides the ones-column
trick for softmax denominators (head B offset so both normalize multiplies
stay in-lane).
"""

import numpy as np
import ml_dtypes

import concourse.mybir as mybir
import concourse.tile as tile
from concourse import bacc
from concourse.bass_utils import run_bass_kernel_spmd

S, E, H, DK, DV = 2048, 1024, 16, 64, 64
NCORES = 8
HPC = H // NCORES          # heads per core = 2
CSL = HPC * DV             # concat-dim columns per core = 128
P = 128
NE = E // P                # 8 contraction chunks for projections
SQB = 512                  # sequence block (PSUM-bank-limited matmul width)
NSQB = S // SQB            # 4
NCH = S // P               # 16 sk chunks of 128
F32 = mybir.dt.float32
BF16 = mybir.dt.bfloat16
SCALE = 1.0 / np.sqrt(DK).astype(np.float32)  # 1/8

EXP = mybir.ActivationFunctionType.Exp
MULT = mybir.AluOpType.mult

_cache = {}
last_results = None  # BassKernelResults of the most recent run (for test.py)
TRACE = False


def _build_nc():
    nc = bacc.Bacc("TRN2", target_bir_lowering=False, debug=False)

    xT = nc.dram_tensor("xT", [P, NSQB, NE, SQB], BF16, kind="ExternalInput")
    wqT = nc.dram_tensor("wqT", [P, NE, CSL], BF16, kind="ExternalInput")
    wkT = nc.dram_tensor("wkT", [P, NE, CSL], BF16, kind="ExternalInput")
    wvT = nc.dram_tensor("wvT", [P, NE, CSL], BF16, kind="ExternalInput")
    woT = nc.dram_tensor("woT", [CSL, E], BF16, kind="ExternalInput")
    ident = nc.dram_tensor("ident", [P, P], BF16, kind="ExternalInput")
    y = nc.dram_tensor("y", [NCH, P, E], BF16, kind="ExternalOutput")

    xT_r = xT.ap()
    w_r = {"q": wqT.ap(), "k": wkT.ap(), "v": wvT.ap()}
    y_ap = y.ap()

    with tile.TileContext(nc) as tc:
        with tc.tile_pool(name="persist", bufs=1) as persist, \
             tc.tile_pool(name="xw", bufs=1) as xw:
            qt = persist.tile([P, S], BF16)          # QT, both heads stacked
            kpad = [
                persist.tile([P, S], BF16, name=f"kpad{h}", tag=f"kpad{h}")
                for h in range(HPC)
            ]
            vt = persist.tile([P, S], BF16, name="vt", tag="vt")
            # head A V-block: [V(64) | ones(2)]; head B: [32 zeros | ones(2) |
            # 30 zeros | V]: its attention output lands on partitions 64-127
            # and its denominators on 32-33 (32-aligned for custom-DVE reads)
            vaug0 = persist.tile([P, NCH, DV + 2], BF16, name="vaug0", tag="vaug0")
            vaug1 = persist.tile([P, NCH, P], BF16, name="vaug1", tag="vaug1")
            wosb = persist.tile([P, E], BF16)
            idsb = persist.tile([P, P], BF16, name="idsb", tag="idsb")
            warmsb = persist.tile([P, SQB], BF16, name="warmsb", tag="warmsb")

            # ---- DMA issue first: 2 HWDGE queues, big transfers.
            # First quarter is split across both queues to land earliest.
            wsb = {}
            for m in ("k", "q", "v"):
                wsb[m] = xw.tile([P, NE, CSL], BF16, name=f"w{m}sb", tag=f"w{m}")
            xq = [
                xw.tile([P, NE, SQB], BF16, name=f"xq{t}", tag=f"xq{t}")
                for t in range(NSQB)
            ]
            nc.sync.dma_start(xq[0][:, 0:4, :], xT_r[:, 0, 0:4, :])
            nc.scalar.dma_start(wsb["k"][:], w_r["k"][:])
            nc.scalar.dma_start(wsb["q"][:], w_r["q"][:])
            nc.scalar.dma_start(xq[0][:, 4:8, :], xT_r[:, 0, 4:8, :])
            nc.sync.dma_start(xq[1][:], xT_r[:, 1])
            nc.scalar.dma_start(wsb["v"][:], w_r["v"][:])
            nc.scalar.dma_start(xq[2][:], xT_r[:, 2])
            nc.sync.dma_start(idsb[:], ident.ap())
            nc.sync.dma_start(xq[3][:], xT_r[:, 3])
            nc.sync.dma_start(wosb[:], woT.ap())

            # warm the ACT exp table set right after the DMA dispatches
            warm = persist.tile([1, 16], F32, name="warm", tag="warm")
            warm2 = persist.tile([1, 16], F32, name="warm2", tag="warm2")
            nc.gpsimd.memset(warmsb[:], 0.25)
            nc.gpsimd.memset(warm[:], 0.0)
            nc.scalar.activation(warm2[:], warm[:], EXP)

            # zero/one fills (gpsimd, no DMA duties this kernel)
            nc.gpsimd.memset(kpad[0][DK:P, :], 0.0)
            nc.gpsimd.memset(kpad[1][0:DK, :], 0.0)
            nc.gpsimd.memset(vaug0[:, :, DV : DV + 2], 1.0)
            nc.gpsimd.memset(vaug1[:, :, 0:32], 0.0)
            nc.gpsimd.memset(vaug1[:, :, 32:34], 1.0)
            nc.gpsimd.memset(vaug1[:, :, 34:DV], 0.0)

            with tc.tile_pool(name="ps", bufs=1, space="PSUM") as ps, \
                 tc.tile_pool(name="est", bufs=8) as est_pool, \
                 tc.tile_pool(name="a1t", bufs=2) as a1t_pool, \
                 tc.tile_pool(name="small", bufs=6) as small, \
                 tc.tile_pool(name="outp", bufs=4) as outp:

                # ---- PE warm-up: junk matmuls from t~0 so the HAM
                # un-throttles before the first projection ----
                wps = ps.tile([P, 2 * SQB], F32, name="wps", tag="sc", bufs=2)
                for i in range(16):
                    nc.tensor.matmul(
                        wps[:, 0:SQB], lhsT=warmsb[:, 0:P], rhs=warmsb[:],
                        start=True, stop=True,
                    )

                # ---- projections (per quarter) ----
                def proj_kq(t, which):
                    sl = slice(t * SQB, (t + 1) * SQB)
                    pp = ps.tile(
                        [P, SQB], F32, name=f"p{which}{t}", tag="kq", bufs=1
                    )
                    for n in range(NE):
                        nc.tensor.matmul(
                            pp[:], lhsT=wsb[which][:, n, :], rhs=xq[t][:, n, :],
                            start=(n == 0), stop=(n == NE - 1),
                        )
                    if which == "q":
                        nc.vector.tensor_copy(qt[:, sl], pp[:])
                    else:
                        nc.vector.tensor_copy(kpad[0][0:DK, sl], pp[0:DK, :])
                        nc.vector.tensor_copy(kpad[1][DK:P, sl], pp[DK:P, :])

                def proj_vt(t):
                    sl = slice(t * SQB, (t + 1) * SQB)
                    pv = ps.tile([P, SQB], F32, name=f"pv{t}", tag="v", bufs=1)
                    for n in range(NE):
                        nc.tensor.matmul(
                            pv[:], lhsT=wsb["v"][:, n, :], rhs=xq[t][:, n, :],
                            start=(n == 0), stop=(n == NE - 1),
                        )
                    nc.vector.tensor_copy(vt[:, sl], pv[:])

                def transp_v(t):
                    vp = ps.tile(
                        [P, 4, P], BF16, name=f"vp{t}", tag="kq", bufs=1
                    )
                    for j in range(4):
                        c = 4 * t + j
                        nc.tensor.transpose(
                            vp[:, j, :], vt[:, c * P : (c + 1) * P], idsb[:]
                        )
                        nc.vector.tensor_copy(vaug0[:, c, 0:DV], vp[:, j, 0:DV])
                        nc.vector.tensor_copy(vaug1[:, c, DV:P], vp[:, j, DV:P])

                # ---- attention granules: (block b, group g of 2 chunks,
                # head h). scores -> exp (ACT) -> AV accumulate ----
                def score_granule(b, g, h):
                    bsl = slice(b * SQB, (b + 1) * SQB)
                    pss = ps.tile(
                        [P, 2 * SQB], F32, name=f"ss{b}_{g}_{h}", tag="sc",
                        bufs=2,
                    )
                    for j in range(2):
                        c = 2 * g + j
                        nc.tensor.matmul(
                            pss[:, j * SQB : (j + 1) * SQB],
                            lhsT=kpad[h][:, c * P : (c + 1) * P],
                            rhs=qt[:, bsl],
                            start=True, stop=True,
                        )
                    es = est_pool.tile(
                        [P, 2 * SQB], BF16, name=f"es{b}_{g}_{h}", tag="est"
                    )
                    nc.scalar.activation(es[:], pss[:], EXP, scale=float(SCALE))
                    return es

                def av_granule(g, h, at_ps, es):
                    for j in range(2):
                        c = 2 * g + j
                        if h == 0:
                            nc.tensor.matmul(
                                at_ps[0 : DV + 2, :],
                                lhsT=vaug0[:, c, :],
                                rhs=es[:, j * SQB : (j + 1) * SQB],
                                start=(c == 0), stop=(c == NCH - 1),
                            )
                        else:
                            nc.tensor.matmul(
                                at_ps[:],
                                lhsT=vaug1[:, c, :],
                                rhs=es[:, j * SQB : (j + 1) * SQB],
                                start=(c == 0), stop=(c == NCH - 1),
                            )

                def norm_head(b, at_ps, a1t, h, last):
                    # normalize: A1T rows = A^T * (1/rowsum); head A rows 0-63
                    # (denoms at 64), head B rows 64-127 (denoms at 32)
                    src = at_ps[h][DV : DV + 1, :] if h == 0 else at_ps[h][32:33, :]
                    rs = small.tile([1, SQB], F32, tag=f"rs{h}")
                    if last:
                        nc.scalar.copy(rs[:], src)  # ACT is idle at the end
                    else:
                        nc.vector.tensor_copy(rs[:], src)
                    rsr = small.tile([1, SQB], F32, tag=f"rsr{h}")
                    nc.vector.reciprocal_approx_fast(rsr[:], rs[:])
                    bc = small.tile([P, SQB], F32, tag=f"bc{h}")
                    nc.gpsimd.partition_broadcast(bc[:], rsr[:])
                    rows = slice(0, DV) if h == 0 else slice(DV, P)
                    nc.vector.tensor_tensor(
                        a1t[rows, :], at_ps[h][rows, :], bc[rows, :], MULT
                    )

                def finish_block(b, at_ps, last):
                    a1t = a1t_pool.tile([P, SQB], BF16, name=f"a1t{b}", tag="a1t")
                    norm_head(b, at_ps, a1t, 0, last)
                    if last:
                        flush_one()  # final head-B AV overlaps head-A chain
                    norm_head(b, at_ps, a1t, 1, last)

                    # output projection for this block: psum borrows the proj
                    # banks (kq/v tags; + sc for the last block), y DMA per
                    # 128-row chunk on the sync queue
                    for j in range(NSQB):
                        osb = outp.tile([P, E], BF16, tag="osb")
                        for e2 in range(E // SQB):
                            esl = slice(e2 * SQB, (e2 + 1) * SQB)
                            if last and e2 == 0:
                                ops = ps.tile(
                                    [P, SQB], F32, name=f"op{b}_{j}_{e2}",
                                    tag="sc", bufs=2,
                                )
                            else:
                                ops = ps.tile(
                                    [P, SQB], F32, name=f"op{b}_{j}_{e2}",
                                    tag=("kq" if e2 == 0 else "v"), bufs=1,
                                )
                            nc.tensor.matmul(
                                ops[:],
                                lhsT=a1t[:, j * P : (j + 1) * P],
                                rhs=wosb[:, esl],
                                start=True, stop=True,
                            )
                            if last and e2 == 0:
                                # ScalarE is idle after the last exp
                                nc.scalar.copy(osb[:, esl], ops[:])
                            else:
                                nc.vector.tensor_copy(osb[:, esl], ops[:])
                        nc.sync.dma_start(y_ap[NSQB * b + j, :, :], osb[:])

                # ---- phase 1: quarters + block-0 attention interleaved ----
                from collections import deque

                at_tiles = {}
                at_tiles[0] = [
                    ps.tile([P, SQB], F32, name=f"at0_{h}", tag="av", bufs=2)
                    for h in range(HPC)
                ]
                pend = deque()  # (b, g, h, es) awaiting AV emission

                def emit_scores(b, g, h):
                    pend.append((b, g, h, score_granule(b, g, h)))

                def flush_one():
                    pb, pg, ph, pes = pend.popleft()
                    av_granule(pg, ph, at_tiles[pb][ph], pes)

                def emit_flush(b, g, h):
                    # steady state: emit scores granule i+1, then AV of i
                    emit_scores(b, g, h)
                    while len(pend) > 1:
                        flush_one()

                for t in range(NSQB):
                    proj_kq(t, "k")
                    if t == 0:
                        proj_kq(0, "q")
                    emit_flush(0, 2 * t, 0)
                    # no flush: av of (2t, 0) must wait for this quarter's
                    # V transposes (it reads vaug chunks 4t, 4t+1)
                    emit_scores(0, 2 * t, 1)
                    proj_vt(t)
                    transp_v(t)
                    emit_flush(0, 2 * t + 1, 0)
                    emit_flush(0, 2 * t + 1, 1)
                    if t == 1:
                        # Q2/Q3 are projected inside blocks 1/2 where the PE
                        # has slack; Q1 here (block 1 starts right after)
                        proj_kq(1, "q")
                    # pend leaves each quarter with exactly one entry

                # ---- phase 2: blocks 1..3. finish of block b-1 is emitted
                # after three score granules of block b (all its at-psum
                # reads must precede block b's first AV write, which reuses
                # the same psum slots), so its outproj fills the PE while
                # ACT churns through the queued exps ----
                for b in range(1, NSQB):
                    emit_scores(b, 0, 0)
                    flush_one()  # av of (b-1, 7, 1): completes block b-1
                    emit_scores(b, 0, 1)
                    emit_scores(b, 1, 0)
                    emit_scores(b, 1, 1)  # 4 exps queued before finish
                    finish_block(b - 1, at_tiles[b - 1], last=False)
                    if b + 1 < NSQB:
                        proj_kq(b + 1, "q")
                    # allocate AFTER finish_block so the psum-slot reuse
                    # sees the normalize reads of block b-1
                    at_tiles[b] = [
                        ps.tile(
                            [P, SQB], F32, name=f"at{b}_{h}", tag="av", bufs=2
                        )
                        for h in range(HPC)
                    ]
                    flush_one()  # av of (b, 0, 0)
                    flush_one()  # av of (b, 0, 1)
                    flush_one()  # av of (b, 1, 0)
                    for g in range(2, NCH // 2):
                        for h in range(HPC):
                            emit_flush(b, g, h)
                # pend holds (3, 7, 1); finish_block(last=True) flushes it
                # between the two head-normalize chains
                finish_block(NSQB - 1, at_tiles[NSQB - 1], last=True)

    nc.compile()
    return nc


def kernel(x, Wq, Wk, Wv, Wo):
    global last_results
    x = np.asarray(x, dtype=np.float32)
    Wq = np.asarray(Wq, dtype=np.float32)
    Wk = np.asarray(Wk, dtype=np.float32)
    Wv = np.asarray(Wv, dtype=np.float32)
    Wo = np.asarray(Wo, dtype=np.float32)

    if "nc" not in _cache:
        _cache["nc"] = _build_nc()
    nc = _cache["nc"]

    bf = ml_dtypes.bfloat16
    # [S, E] -> [P, NSQB, NE, SQB]: xT[p, t, n, s] = x[t*SQB+s, n*P+p]
    xTq = np.ascontiguousarray(
        x.reshape(NSQB, SQB, NE, P).transpose(3, 0, 2, 1).astype(bf)
    )
    WqT = np.ascontiguousarray(Wq.T)
    WkT = np.ascontiguousarray(Wk.T)
    WvT = np.ascontiguousarray(Wv.T)
    WoT = np.ascontiguousarray(Wo.T)

    in_maps = []
    for i in range(NCORES):
        sl = slice(i * CSL, (i + 1) * CSL)

        def wslice(WT):
            # [E, CSL] slice -> [P, NE, CSL] partition-major
            return np.ascontiguousarray(
                WT[:, sl].reshape(NE, P, CSL).transpose(1, 0, 2).astype(bf)
            )

        in_maps.append({
            "xT": xTq,
            "ident": np.eye(P, dtype=np.float32).astype(bf),
            "wqT": wslice(WqT),
            "wkT": wslice(WkT),
            "wvT": wslice(WvT),
            "woT": np.ascontiguousarray(WoT[sl, :].astype(bf)),
        })

    last_results = run_bass_kernel_spmd(
        nc, in_maps, core_ids=list(range(NCORES)), trace=TRACE
    )
    out = np.zeros((S, E), dtype=np.float32)
    for r in last_results.results:
        out += r["y"].astype(np.float32).reshape(S, E)
    return out


# revision 17
# speedup vs baseline: 1.1223x; 1.1223x over previous
"""Multi-head attention TRN2 Bass kernel, head-sharded across 8 NeuronCores.

Problem: S=2048, E=1024, H=16 heads, dk=dv=64, fp32.
    Q = x @ Wq.T ; K = x @ Wk.T ; V = x @ Wv.T   (per-head slices)
    A_h = softmax(Q_h K_h^T / 8) V_h
    out = concat_h(A_h) @ Wo.T
Sharding: tensor-parallel over heads. Core i owns heads (2i, 2i+1); the 8
partial [2048,1024] outputs are summed on the host.

v6 layout (per-core):
  * Few, large HWDGE input transfers; first x quarter split across both
    queues so the first projection starts earliest. All y output DMAs ride
    the sync queue only -- a dma_start costs ~0.6us of issuing-engine time
    and the scalar engine (exp) is the kernel bottleneck.
  * 16 warm-up matmuls at t~0 open the HAM clock gate before real work.
  * Per-quarter pipeline: K_t -> scores granule -> V_t^T -> PE transposes ->
    scores granule -> Q_t, with block-0 attention interleaved so exp starts
    as soon as quarter 0 lands.
  * finish_block(b) (normalize + output projection) is emitted in the middle
    of block b+1 so its PE work fills the scores/AV pipeline instead of
    draining ACT at block boundaries.
  * PSUM: scores 2x[128,1024] + AV accum 2x[128,512] + kq 1 + v 1 = 8 banks.
  * y written per 128-row chunk as one [128,1024] DMA into a [16,128,E]
    DRAM layout (host reshape is free).
All matmul operands bf16 (fp32 PSUM accumulation). AV rides the ones-column
trick for softmax denominators (head B offset so both normalize multiplies
stay in-lane).
"""

import numpy as np
import ml_dtypes

import concourse.mybir as mybir
import concourse.tile as tile
from concourse import bacc
from concourse.bass_utils import run_bass_kernel_spmd

S, E, H, DK, DV = 2048, 1024, 16, 64, 64
NCORES = 8
HPC = H // NCORES          # heads per core = 2
CSL = HPC * DV             # concat-dim columns per core = 128
P = 128
NE = E // P                # 8 contraction chunks for projections
SQB = 512                  # sequence block (PSUM-bank-limited matmul width)
NSQB = S // SQB            # 4
NCH = S // P               # 16 sk chunks of 128
F32 = mybir.dt.float32
BF16 = mybir.dt.bfloat16
SCALE = 1.0 / np.sqrt(DK).astype(np.float32)  # 1/8

EXP = mybir.ActivationFunctionType.Exp
MULT = mybir.AluOpType.mult

_cache = {}
last_results = None  # BassKernelResults of the most recent run (for test.py)
TRACE = False


def _build_nc():
    nc = bacc.Bacc("TRN2", target_bir_lowering=False, debug=False)

    xT = nc.dram_tensor("xT", [P, NSQB, NE, SQB], BF16, kind="ExternalInput")
    wqT = nc.dram_tensor("wqT", [P, NE, CSL], BF16, kind="ExternalInput")
    wkT = nc.dram_tensor("wkT", [P, NE, CSL], BF16, kind="ExternalInput")
    wvT = nc.dram_tensor("wvT", [P, NE, CSL], BF16, kind="ExternalInput")
    woT = nc.dram_tensor("woT", [CSL, E], BF16, kind="ExternalInput")
    ident = nc.dram_tensor("ident", [P, P], BF16, kind="ExternalInput")
    y = nc.dram_tensor("y", [NCH, P, E], BF16, kind="ExternalOutput")

    xT_r = xT.ap()
    w_r = {"q": wqT.ap(), "k": wkT.ap(), "v": wvT.ap()}
    y_ap = y.ap()

    with tile.TileContext(nc) as tc:
        with tc.tile_pool(name="persist", bufs=1) as persist, \
             tc.tile_pool(name="xw", bufs=1) as xw:
            qt = persist.tile([P, S], BF16)          # QT, both heads stacked
            kpad = [
                persist.tile([P, S], BF16, name=f"kpad{h}", tag=f"kpad{h}")
                for h in range(HPC)
            ]
            vt = persist.tile([P, S], BF16, name="vt", tag="vt")
            # head A V-block: [V(64) | ones(2)]; head B: [32 zeros | ones(2) |
            # 30 zeros | V]: its attention output lands on partitions 64-127
            # and its denominators on 32-33 (32-aligned for custom-DVE reads)
            vaug0 = persist.tile([P, NCH, DV + 2], BF16, name="vaug0", tag="vaug0")
            vaug1 = persist.tile([P, NCH, P], BF16, name="vaug1", tag="vaug1")
            wosb = persist.tile([P, E], BF16)
            idsb = persist.tile([P, P], BF16, name="idsb", tag="idsb")
            warmsb = persist.tile([P, SQB], BF16, name="warmsb", tag="warmsb")

            # ---- DMA issue first: 2 HWDGE queues, big transfers.
            # First quarter is split across both queues to land earliest.
            wsb = {}
            for m in ("k", "q", "v"):
                wsb[m] = xw.tile([P, NE, CSL], BF16, name=f"w{m}sb", tag=f"w{m}")
            xq = [
                xw.tile([P, NE, SQB], BF16, name=f"xq{t}", tag=f"xq{t}")
                for t in range(NSQB)
            ]
            nc.sync.dma_start(xq[0][:, 0:4, :], xT_r[:, 0, 0:4, :])
            nc.scalar.dma_start(wsb["k"][:], w_r["k"][:])
            nc.scalar.dma_start(wsb["q"][:], w_r["q"][:])
            nc.scalar.dma_start(xq[0][:, 4:8, :], xT_r[:, 0, 4:8, :])
            nc.sync.dma_start(xq[1][:], xT_r[:, 1])
            nc.scalar.dma_start(wsb["v"][:], w_r["v"][:])
            nc.scalar.dma_start(xq[2][:], xT_r[:, 2])
            nc.sync.dma_start(idsb[:], ident.ap())
            nc.sync.dma_start(xq[3][:], xT_r[:, 3])
            nc.sync.dma_start(wosb[:], woT.ap())

            # warm the ACT exp table set right after the DMA dispatches
            warm = persist.tile([1, 16], F32, name="warm", tag="warm")
            warm2 = persist.tile([1, 16], F32, name="warm2", tag="warm2")
            nc.gpsimd.memset(warmsb[:], 0.25)
            nc.gpsimd.memset(warm[:], 0.0)
            nc.scalar.activation(warm2[:], warm[:], EXP)

            # zero/one fills (gpsimd, no DMA duties this kernel)
            nc.gpsimd.memset(kpad[0][DK:P, :], 0.0)
            nc.gpsimd.memset(kpad[1][0:DK, :], 0.0)
            nc.gpsimd.memset(vaug0[:, :, DV : DV + 2], 1.0)
            nc.gpsimd.memset(vaug1[:, :, 0:32], 0.0)
            nc.gpsimd.memset(vaug1[:, :, 32:34], 1.0)
            nc.gpsimd.memset(vaug1[:, :, 34:DV], 0.0)

            with tc.tile_pool(name="ps", bufs=1, space="PSUM") as ps, \
                 tc.tile_pool(name="est", bufs=8) as est_pool, \
                 tc.tile_pool(name="a1t", bufs=2) as a1t_pool, \
                 tc.tile_pool(name="small", bufs=6) as small, \
                 tc.tile_pool(name="outp", bufs=4) as outp:

                # ---- PE warm-up: junk matmuls from t~0 so the HAM
                # un-throttles before the first projection ----
                wps = ps.tile([P, 2 * SQB], F32, name="wps", tag="sc", bufs=2)
                for i in range(16):
                    nc.tensor.matmul(
                        wps[:, 0:SQB], lhsT=warmsb[:, 0:P], rhs=warmsb[:],
                        start=True, stop=True,
                    )

                # ---- projections (per quarter) ----
                def proj_kq(t, which):
                    sl = slice(t * SQB, (t + 1) * SQB)
                    pp = ps.tile(
                        [P, SQB], F32, name=f"p{which}{t}", tag="kq", bufs=1
                    )
                    for n in range(NE):
                        nc.tensor.matmul(
                            pp[:], lhsT=wsb[which][:, n, :], rhs=xq[t][:, n, :],
                            start=(n == 0), stop=(n == NE - 1),
                        )
                    if which == "q":
                        nc.vector.tensor_copy(qt[:, sl], pp[:])
                    else:
                        nc.vector.tensor_copy(kpad[0][0:DK, sl], pp[0:DK, :])
                        nc.vector.tensor_copy(kpad[1][DK:P, sl], pp[DK:P, :])

                def proj_vt(t):
                    sl = slice(t * SQB, (t + 1) * SQB)
                    pv = ps.tile([P, SQB], F32, name=f"pv{t}", tag="v", bufs=1)
                    for n in range(NE):
                        nc.tensor.matmul(
                            pv[:], lhsT=wsb["v"][:, n, :], rhs=xq[t][:, n, :],
                            start=(n == 0), stop=(n == NE - 1),
                        )
                    nc.vector.tensor_copy(vt[:, sl], pv[:])

                def transp_v(t):
                    vp = ps.tile(
                        [P, 4, P], BF16, name=f"vp{t}", tag="kq", bufs=1
                    )
                    for j in range(4):
                        c = 4 * t + j
                        nc.tensor.transpose(
                            vp[:, j, :], vt[:, c * P : (c + 1) * P], idsb[:]
                        )
                        nc.vector.tensor_copy(vaug0[:, c, 0:DV], vp[:, j, 0:DV])
                        nc.vector.tensor_copy(vaug1[:, c, DV:P], vp[:, j, DV:P])

                # ---- attention granules: (block b, group g of 2 chunks,
                # head h). scores -> exp (ACT) -> AV accumulate ----
                def score_granule(b, g, h):
                    bsl = slice(b * SQB, (b + 1) * SQB)
                    pss = ps.tile(
                        [P, 2 * SQB], F32, name=f"ss{b}_{g}_{h}", tag="sc",
                        bufs=2,
                    )
                    for j in range(2):
                        c = 2 * g + j
                        nc.tensor.matmul(
                            pss[:, j * SQB : (j + 1) * SQB],
                            lhsT=kpad[h][:, c * P : (c + 1) * P],
                            rhs=qt[:, bsl],
                            start=True, stop=True,
                        )
                    es = est_pool.tile(
                        [P, 2 * SQB], BF16, name=f"es{b}_{g}_{h}", tag="est"
                    )
                    nc.scalar.activation(es[:], pss[:], EXP, scale=float(SCALE))
                    return es

                def av_granule(g, h, at_ps, es):
                    for j in range(2):
                        c = 2 * g + j
                        if h == 0:
                            nc.tensor.matmul(
                                at_ps[0 : DV + 2, :],
                                lhsT=vaug0[:, c, :],
                                rhs=es[:, j * SQB : (j + 1) * SQB],
                                start=(c == 0), stop=(c == NCH - 1),
                            )
                        else:
                            nc.tensor.matmul(
                                at_ps[:],
                                lhsT=vaug1[:, c, :],
                                rhs=es[:, j * SQB : (j + 1) * SQB],
                                start=(c == 0), stop=(c == NCH - 1),
                            )

                def norm_head(b, at_ps, a1t, h, last):
                    # normalize: A1T rows = A^T * (1/rowsum); head A rows 0-63
                    # (denoms at 64), head B rows 64-127 (denoms at 32)
                    src = at_ps[h][DV : DV + 1, :] if h == 0 else at_ps[h][32:33, :]
                    rs = small.tile([1, SQB], F32, tag=f"rs{h}")
                    if last:
                        nc.scalar.copy(rs[:], src)  # ACT is idle at the end
                    else:
                        nc.vector.tensor_copy(rs[:], src)
                    rsr = small.tile([1, SQB], F32, tag=f"rsr{h}")
                    nc.vector.reciprocal_approx_fast(rsr[:], rs[:])
                    bc = small.tile([P, SQB], F32, tag=f"bc{h}")
                    nc.gpsimd.partition_broadcast(bc[:], rsr[:])
                    rows = slice(0, DV) if h == 0 else slice(DV, P)
                    nc.vector.tensor_tensor(
                        a1t[rows, :], at_ps[h][rows, :], bc[rows, :], MULT
                    )

                def norm_block(b, at_ps, last):
                    a1t = a1t_pool.tile([P, SQB], BF16, name=f"a1t{b}", tag="a1t")
                    norm_head(b, at_ps, a1t, 0, last)
                    if last:
                        flush_one()  # final head-B AV overlaps head-A chain
                    norm_head(b, at_ps, a1t, 1, last)
                    return a1t

                def outproj_j(b, a1t, j, last):
                    # output projection for rows [b*512+j*128, ..+128): psum
                    # borrows the proj banks (kq/v; + sc for the last block),
                    # y DMA per 128-row chunk on the sync queue
                    osb = outp.tile([P, E], BF16, tag="osb")
                    for e2 in range(E // SQB):
                        esl = slice(e2 * SQB, (e2 + 1) * SQB)
                        if last and e2 == 0:
                            ops = ps.tile(
                                [P, SQB], F32, name=f"op{b}_{j}_{e2}",
                                tag="sc", bufs=2,
                            )
                        else:
                            ops = ps.tile(
                                [P, SQB], F32, name=f"op{b}_{j}_{e2}",
                                tag=("kq" if e2 == 0 else "v"), bufs=1,
                            )
                        nc.tensor.matmul(
                            ops[:],
                            lhsT=a1t[:, j * P : (j + 1) * P],
                            rhs=wosb[:, esl],
                            start=True, stop=True,
                        )
                        if last and e2 == 0:
                            # ScalarE is idle after the last exp
                            nc.scalar.copy(osb[:, esl], ops[:])
                        else:
                            nc.vector.tensor_copy(osb[:, esl], ops[:])
                    nc.sync.dma_start(y_ap[NSQB * b + j, :, :], osb[:])

                # ---- phase 1: quarters + block-0 attention interleaved ----
                from collections import deque

                at_tiles = {}
                at_tiles[0] = [
                    ps.tile([P, SQB], F32, name=f"at0_{h}", tag="av", bufs=2)
                    for h in range(HPC)
                ]
                pend = deque()  # (b, g, h, es) awaiting AV emission

                def emit_scores(b, g, h):
                    pend.append((b, g, h, score_granule(b, g, h)))

                def flush_one():
                    pb, pg, ph, pes = pend.popleft()
                    av_granule(pg, ph, at_tiles[pb][ph], pes)

                def emit_flush(b, g, h):
                    # steady state: emit scores granule i+1, then AV of i
                    emit_scores(b, g, h)
                    while len(pend) > 1:
                        flush_one()

                for t in range(NSQB):
                    proj_kq(t, "k")
                    if t == 0:
                        proj_kq(0, "q")
                    emit_flush(0, 2 * t, 0)
                    # no flush: av of (2t, 0) must wait for this quarter's
                    # V transposes (it reads vaug chunks 4t, 4t+1)
                    emit_scores(0, 2 * t, 1)
                    proj_vt(t)
                    transp_v(t)
                    emit_flush(0, 2 * t + 1, 0)
                    emit_flush(0, 2 * t + 1, 1)
                    if t > 0:
                        proj_kq(t, "q")
                    # pend leaves each quarter with exactly one entry

                # ---- phase 2: blocks 1..3. finish of block b-1 is emitted
                # after three score granules of block b (all its at-psum
                # reads must precede block b's first AV write, which reuses
                # the same psum slots), so its outproj fills the PE while
                # ACT churns through the queued exps ----
                for b in range(1, NSQB):
                    emit_scores(b, 0, 0)
                    flush_one()  # av of (b-1, 7, 1): completes block b-1
                    emit_scores(b, 0, 1)
                    emit_scores(b, 1, 0)
                    a1t_prev = norm_block(b - 1, at_tiles[b - 1], last=False)
                    # allocate AFTER norm_block so the psum-slot reuse sees
                    # the normalize reads of block b-1
                    at_tiles[b] = [
                        ps.tile(
                            [P, SQB], F32, name=f"at{b}_{h}", tag="av", bufs=2
                        )
                        for h in range(HPC)
                    ]
                    # interleave block b-1's output projection with block b's
                    # score/AV beats so ACT never drains
                    outproj_j(b - 1, a1t_prev, 0, last=False)
                    flush_one()  # av of (b, 0, 0)
                    outproj_j(b - 1, a1t_prev, 1, last=False)
                    emit_scores(b, 1, 1)
                    flush_one()  # av of (b, 0, 1)
                    outproj_j(b - 1, a1t_prev, 2, last=False)
                    emit_scores(b, 2, 0)
                    flush_one()  # av of (b, 1, 0)
                    outproj_j(b - 1, a1t_prev, 3, last=False)
                    emit_scores(b, 2, 1)
                    flush_one()  # av of (b, 1, 1)
                    for g in range(3, NCH // 2):
                        for h in range(HPC):
                            emit_flush(b, g, h)
                # pend holds (3, 7, 1); norm_block(last=True) flushes it
                # between the two head-normalize chains
                a1t_last = norm_block(NSQB - 1, at_tiles[NSQB - 1], last=True)
                for j in range(NSQB):
                    outproj_j(NSQB - 1, a1t_last, j, last=True)

    nc.compile()
    return nc


def kernel(x, Wq, Wk, Wv, Wo):
    global last_results
    x = np.asarray(x, dtype=np.float32)
    Wq = np.asarray(Wq, dtype=np.float32)
    Wk = np.asarray(Wk, dtype=np.float32)
    Wv = np.asarray(Wv, dtype=np.float32)
    Wo = np.asarray(Wo, dtype=np.float32)

    if "nc" not in _cache:
        _cache["nc"] = _build_nc()
    nc = _cache["nc"]

    bf = ml_dtypes.bfloat16
    # [S, E] -> [P, NSQB, NE, SQB]: xT[p, t, n, s] = x[t*SQB+s, n*P+p]
    xTq = np.ascontiguousarray(
        x.reshape(NSQB, SQB, NE, P).transpose(3, 0, 2, 1).astype(bf)
    )
    WqT = np.ascontiguousarray(Wq.T)
    WkT = np.ascontiguousarray(Wk.T)
    WvT = np.ascontiguousarray(Wv.T)
    WoT = np.ascontiguousarray(Wo.T)

    in_maps = []
    for i in range(NCORES):
        sl = slice(i * CSL, (i + 1) * CSL)

        def wslice(WT):
            # [E, CSL] slice -> [P, NE, CSL] partition-major
            return np.ascontiguousarray(
                WT[:, sl].reshape(NE, P, CSL).transpose(1, 0, 2).astype(bf)
            )

        in_maps.append({
            "xT": xTq,
            "ident": np.eye(P, dtype=np.float32).astype(bf),
            "wqT": wslice(WqT),
            "wkT": wslice(WkT),
            "wvT": wslice(WvT),
            "woT": np.ascontiguousarray(WoT[sl, :].astype(bf)),
        })

    last_results = run_bass_kernel_spmd(
        nc, in_maps, core_ids=list(range(NCORES)), trace=TRACE
    )
    out = np.zeros((S, E), dtype=np.float32)
    for r in last_results.results:
        out += r["y"].astype(np.float32).reshape(S, E)
    return out


# revision 19
# speedup vs baseline: 1.2284x; 1.0946x over previous
"""Multi-head attention TRN2 Bass kernel, head-sharded across 8 NeuronCores.

Problem: S=2048, E=1024, H=16 heads, dk=dv=64, fp32.
    Q = x @ Wq.T ; K = x @ Wk.T ; V = x @ Wv.T   (per-head slices)
    A_h = softmax(Q_h K_h^T / 8) V_h
    out = concat_h(A_h) @ Wo.T
Sharding: tensor-parallel over heads. Core i owns heads (2i, 2i+1); the 8
partial [2048,1024] outputs are summed on the host.

v7 layout (per-core):
  * Scores run as row-tiled K=64 matmul pairs (tile_position (0,0)/(64,0)):
    both heads' score chunks execute CONCURRENTLY in the PE array into
    adjacent PSUM banks, halving score time and killing the zero-padded
    kpad scheme of v4-v6.
  * One exp ACTIVATE per chunk covers both heads ([128, 2x512] psum).
  * Scores run up to a block ahead of their AV consumption (deep es ring):
    while block b's AV/normalize/outproj drains, block b+1's scores keep
    the scalar engine (the kernel bottleneck at ~74us of exp) saturated.
  * Few, large HWDGE input transfers; first x quarter split across both
    queues. y DMAs ride the sync queue only (a dma_start costs ~0.6us of
    issuing-engine time; the scalar engine must spend it on exp instead).
  * Warm-up matmuls at t~0 open the HAM clock gate before real work.
  * PSUM: scores 2x[128,2,512] + AV accum 2x[128,512] + kq 1 + v 1 = 8.
All matmul operands bf16 (fp32 PSUM accumulation). AV rides the ones-column
trick for softmax denominators (head B offset so both normalize multiplies
stay in-lane).
"""

from collections import deque

import numpy as np
import ml_dtypes

import concourse.mybir as mybir
import concourse.tile as tile
from concourse import bacc
from concourse.bass_utils import run_bass_kernel_spmd

S, E, H, DK, DV = 2048, 1024, 16, 64, 64
NCORES = 8
HPC = H // NCORES          # heads per core = 2
CSL = HPC * DV             # concat-dim columns per core = 128
P = 128
NE = E // P                # 8 contraction chunks for projections
SQB = 512                  # sequence block (PSUM-bank-limited matmul width)
NSQB = S // SQB            # 4
NCH = S // P               # 16 sk chunks of 128
F32 = mybir.dt.float32
BF16 = mybir.dt.bfloat16
SCALE = 1.0 / np.sqrt(DK).astype(np.float32)  # 1/8

EXP = mybir.ActivationFunctionType.Exp
MULT = mybir.AluOpType.mult

_cache = {}
last_results = None  # BassKernelResults of the most recent run (for test.py)
TRACE = False


def _build_nc():
    nc = bacc.Bacc("TRN2", target_bir_lowering=False, debug=False)

    xT = nc.dram_tensor("xT", [P, NSQB, NE, SQB], BF16, kind="ExternalInput")
    wqT = nc.dram_tensor("wqT", [P, NE, CSL], BF16, kind="ExternalInput")
    wkT = nc.dram_tensor("wkT", [P, NE, CSL], BF16, kind="ExternalInput")
    wvT = nc.dram_tensor("wvT", [P, NE, CSL], BF16, kind="ExternalInput")
    woT = nc.dram_tensor("woT", [CSL, E], BF16, kind="ExternalInput")
    ident = nc.dram_tensor("ident", [P, P], BF16, kind="ExternalInput")
    y = nc.dram_tensor("y", [NCH, P, E], BF16, kind="ExternalOutput")

    xT_r = xT.ap()
    w_r = {"q": wqT.ap(), "k": wkT.ap(), "v": wvT.ap()}
    y_ap = y.ap()

    with tile.TileContext(nc) as tc:
        with tc.tile_pool(name="persist", bufs=1) as persist, \
             tc.tile_pool(name="xw", bufs=1) as xw:
            qt = persist.tile([P, S], BF16)   # QT, heads on partitions 0-63/64-127
            kt = persist.tile([P, S], BF16)   # KT, same head split
            vt = persist.tile([P, S], BF16, name="vt", tag="vt")
            # head A V-block: [V(64) | ones(2)]; head B: [32 zeros | ones(2) |
            # 30 zeros | V]: its attention output lands on partitions 64-127
            # and its denominators on 32-33 (32-aligned for custom-DVE reads)
            vaug0 = persist.tile([P, NCH, DV + 2], BF16, name="vaug0", tag="vaug0")
            vaug1 = persist.tile([P, NCH, P], BF16, name="vaug1", tag="vaug1")
            wosb = persist.tile([P, E], BF16)
            idsb = persist.tile([P, P], BF16, name="idsb", tag="idsb")
            warmsb = persist.tile([P, SQB], BF16, name="warmsb", tag="warmsb")

            # ---- DMA issue first: 2 HWDGE queues, big transfers ----
            wsb = {}
            for m in ("k", "q", "v"):
                wsb[m] = xw.tile([P, NE, CSL], BF16, name=f"w{m}sb", tag=f"w{m}")
            xq = [
                xw.tile([P, NE, SQB], BF16, name=f"xq{t}", tag=f"xq{t}")
                for t in range(NSQB)
            ]
            nc.sync.dma_start(xq[0][:, 0:4, :], xT_r[:, 0, 0:4, :])
            nc.scalar.dma_start(wsb["k"][:], w_r["k"][:])
            nc.scalar.dma_start(wsb["q"][:], w_r["q"][:])
            nc.scalar.dma_start(xq[0][:, 4:8, :], xT_r[:, 0, 4:8, :])
            nc.sync.dma_start(xq[1][:], xT_r[:, 1])
            nc.scalar.dma_start(wsb["v"][:], w_r["v"][:])
            nc.sync.dma_start(idsb[:], ident.ap())
            nc.scalar.dma_start(xq[2][:], xT_r[:, 2])
            nc.sync.dma_start(xq[3][:], xT_r[:, 3])
            nc.sync.dma_start(wosb[:], woT.ap())

            # warm the ACT exp table set right after the DMA dispatches
            warm = persist.tile([1, 16], F32, name="warm", tag="warm")
            warm2 = persist.tile([1, 16], F32, name="warm2", tag="warm2")
            nc.gpsimd.memset(warmsb[:], 0.25)
            nc.gpsimd.memset(warm[:], 0.0)
            nc.scalar.activation(warm2[:], warm[:], EXP)

            nc.gpsimd.memset(vaug0[:, :, DV : DV + 2], 1.0)
            nc.gpsimd.memset(vaug1[:, :, 0:32], 0.0)
            nc.gpsimd.memset(vaug1[:, :, 32:34], 1.0)
            nc.gpsimd.memset(vaug1[:, :, 34:DV], 0.0)

            with tc.tile_pool(name="ps", bufs=1, space="PSUM") as ps, \
                 tc.tile_pool(name="est", bufs=18) as est_pool, \
                 tc.tile_pool(name="a1t", bufs=2) as a1t_pool, \
                 tc.tile_pool(name="small", bufs=2) as small, \
                 tc.tile_pool(name="outp", bufs=4) as outp:

                # ---- PE warm-up: junk matmuls from t~0 so the HAM
                # un-throttles before the first projection ----
                wps = ps.tile([P, 2, SQB], F32, name="wps", tag="sc", bufs=2)
                for i in range(12):
                    nc.tensor.matmul(
                        wps[:, 0, :], lhsT=warmsb[:, 0:P], rhs=warmsb[:],
                        start=True, stop=True,
                    )

                # ---- projections (per quarter) ----
                def proj(t, which):
                    sl = slice(t * SQB, (t + 1) * SQB)
                    tag = "v" if which == "v" else "kq"
                    pp = ps.tile(
                        [P, SQB], F32, name=f"p{which}{t}", tag=tag, bufs=1
                    )
                    for n in range(NE):
                        nc.tensor.matmul(
                            pp[:], lhsT=wsb[which][:, n, :], rhs=xq[t][:, n, :],
                            start=(n == 0), stop=(n == NE - 1),
                        )
                    dst = {"q": qt, "k": kt, "v": vt}[which]
                    nc.vector.tensor_copy(dst[:, sl], pp[:])

                def transp_v(t):
                    vp = ps.tile(
                        [P, 4, P], BF16, name=f"vp{t}", tag="kq", bufs=1
                    )
                    for j in range(4):
                        c = 4 * t + j
                        nc.tensor.transpose(
                            vp[:, j, :], vt[:, c * P : (c + 1) * P], idsb[:]
                        )
                        nc.vector.tensor_copy(vaug0[:, c, 0:DV], vp[:, j, 0:DV])
                        nc.vector.tensor_copy(vaug1[:, c, DV:P], vp[:, j, DV:P])

                # ---- attention chunk granules: scores for chunk c are a
                # row-tiled concurrent pair (head A rows 0-63, head B rows
                # 64-127), one exp ACTIVATE covers both heads ----
                sq = {b: deque() for b in range(NSQB)}  # (c, es) awaiting AV
                nxt = {b: 0 for b in range(NSQB)}       # next score chunk
                at_tiles = {}

                def s_chunk(b):
                    c = nxt[b]
                    nxt[b] += 1
                    bsl = slice(b * SQB, (b + 1) * SQB)
                    csl = slice(c * P, (c + 1) * P)
                    pss = ps.tile(
                        [P, 2, SQB], F32, name=f"ss{b}_{c}", tag="sc", bufs=2
                    )
                    nc.tensor.matmul(
                        pss[:, 0, :], lhsT=kt[0:DK, csl], rhs=qt[0:DK, bsl],
                        start=True, stop=True, tile_position=(0, 0),
                    )
                    nc.tensor.matmul(
                        pss[:, 1, :], lhsT=kt[DK:P, csl], rhs=qt[DK:P, bsl],
                        start=True, stop=True, tile_position=(64, 0),
                    )
                    es = est_pool.tile(
                        [P, 2, SQB], BF16, name=f"es{b}_{c}", tag="est"
                    )
                    nc.scalar.activation(es[:], pss[:], EXP, scale=float(SCALE))
                    sq[b].append((c, es))

                def av_chunk(b):
                    c, es = sq[b].popleft()
                    at_ps = at_tiles[b]
                    nc.tensor.matmul(
                        at_ps[0][0 : DV + 2, :],
                        lhsT=vaug0[:, c, :], rhs=es[:, 0, :],
                        start=(c == 0), stop=(c == NCH - 1),
                    )
                    nc.tensor.matmul(
                        at_ps[1][:],
                        lhsT=vaug1[:, c, :], rhs=es[:, 1, :],
                        start=(c == 0), stop=(c == NCH - 1),
                    )

                def norm_head(b, a1t, h, last):
                    at_ps = at_tiles[b]
                    src = at_ps[h][DV : DV + 1, :] if h == 0 else at_ps[h][32:33, :]
                    rs = small.tile([1, SQB], F32, tag=f"rs{h}")
                    if last:
                        nc.scalar.copy(rs[:], src)  # ACT is idle at the end
                    else:
                        nc.vector.tensor_copy(rs[:], src)
                    rsr = small.tile([1, SQB], F32, tag=f"rsr{h}")
                    nc.vector.reciprocal_approx_fast(rsr[:], rs[:])
                    bc = small.tile([P, SQB], F32, tag=f"bc{h}")
                    nc.gpsimd.partition_broadcast(bc[:], rsr[:])
                    rows = slice(0, DV) if h == 0 else slice(DV, P)
                    nc.vector.tensor_tensor(
                        a1t[rows, :], at_ps[h][rows, :], bc[rows, :], MULT
                    )

                def norm_block(b, last=False):
                    a1t = a1t_pool.tile([P, SQB], BF16, name=f"a1t{b}", tag="a1t")
                    if last:
                        # split the final chunk's AV by head: head-B's matmul
                        # runs on the PE while head-A's normalize chain
                        # (DVE/gpsimd) is already going
                        c, es = sq[b].popleft()
                        at_ps = at_tiles[b]
                        nc.tensor.matmul(
                            at_ps[0][0 : DV + 2, :],
                            lhsT=vaug0[:, c, :], rhs=es[:, 0, :],
                            start=False, stop=True,
                        )
                        norm_head(b, a1t, 0, last)
                        nc.tensor.matmul(
                            at_ps[1][:],
                            lhsT=vaug1[:, c, :], rhs=es[:, 1, :],
                            start=False, stop=True,
                        )
                        norm_head(b, a1t, 1, last)
                    else:
                        norm_head(b, a1t, 0, last)
                        norm_head(b, a1t, 1, last)
                    return a1t

                def outproj_j(b, a1t, j, last):
                    osb = outp.tile([P, E], BF16, tag="osb")
                    for e2 in range(E // SQB):
                        esl = slice(e2 * SQB, (e2 + 1) * SQB)
                        if last and e2 == 0:
                            ops = ps.tile(
                                [P, SQB], F32, name=f"op{b}_{j}_{e2}",
                                tag="sc", bufs=2,
                            )
                        else:
                            ops = ps.tile(
                                [P, SQB], F32, name=f"op{b}_{j}_{e2}",
                                tag=("kq" if e2 == 0 else "v"), bufs=1,
                            )
                        nc.tensor.matmul(
                            ops[:],
                            lhsT=a1t[:, j * P : (j + 1) * P],
                            rhs=wosb[:, esl],
                            start=True, stop=True,
                        )
                        if last and e2 == 0:
                            nc.scalar.copy(osb[:, esl], ops[:])
                        else:
                            nc.vector.tensor_copy(osb[:, esl], ops[:])
                    nc.sync.dma_start(y_ap[NSQB * b + j, :, :], osb[:])

                # ---- phase 1: quarters, block-0 attention interleaved,
                # block-1 scores as ACT filler once Q1 exists ----
                at_tiles[0] = [
                    ps.tile([P, SQB], F32, name=f"at0_{h}", tag="av", bufs=2)
                    for h in range(HPC)
                ]
                for t in range(NSQB):
                    proj(t, "k")
                    if t == 0:
                        proj(0, "q")
                    if t == 1:
                        proj(1, "q")
                    s_chunk(0)
                    if t > 0:
                        av_chunk(0)
                    s_chunk(0)
                    if t > 0:
                        av_chunk(0)
                    proj(t, "v")
                    transp_v(t)
                    s_chunk(0)
                    av_chunk(0)
                    s_chunk(0)
                    av_chunk(0)
                    # block-1 score-ahead (ACT filler): 4 at t=1, 4 at t=2,
                    # 2 at t=3 -> block 1 enters phase 2 with 10 queued
                    if t == 1 or t == 2:
                        for _ in range(4):
                            s_chunk(1)
                    elif t == 3:
                        for _ in range(2):
                            s_chunk(1)
                # b0 avs lag 2 behind; drained at phase-2 entry

                # ---- phase 2: blocks 1..3 with next-block score-ahead ----
                PRE = 10  # chunks pre-queued for the next block
                for b in range(1, NSQB):
                    while sq[b - 1]:
                        av_chunk(b - 1)
                    a1t_prev = norm_block(b - 1)
                    # allocate AFTER norm_block so the psum-slot reuse sees
                    # the normalize reads of block b-1
                    at_tiles[b] = [
                        ps.tile(
                            [P, SQB], F32, name=f"at{b}_{h}", tag="av", bufs=2
                        )
                        for h in range(HPC)
                    ]
                    if b + 1 < NSQB:
                        proj(b + 1, "q")
                    opj = 0
                    nbeats = NCH if b < NSQB - 1 else NCH - 1
                    for i in range(nbeats):
                        if not sq[b]:
                            s_chunk(b)
                        av_chunk(b)
                        if i % 4 == 1 and opj < NSQB:
                            outproj_j(b - 1, a1t_prev, opj, last=False)
                            opj += 1
                        if nxt[b] < NCH:
                            s_chunk(b)
                        elif b + 1 < NSQB and nxt[b + 1] < PRE:
                            s_chunk(b + 1)
                # last block: final AV overlaps the head-A normalize chain
                a1t_last = norm_block(NSQB - 1, last=True)
                for j in range(NSQB):
                    outproj_j(NSQB - 1, a1t_last, j, last=True)

    nc.compile()
    return nc


def kernel(x, Wq, Wk, Wv, Wo):
    global last_results
    x = np.asarray(x, dtype=np.float32)
    Wq = np.asarray(Wq, dtype=np.float32)
    Wk = np.asarray(Wk, dtype=np.float32)
    Wv = np.asarray(Wv, dtype=np.float32)
    Wo = np.asarray(Wo, dtype=np.float32)

    if "nc" not in _cache:
        _cache["nc"] = _build_nc()
    nc = _cache["nc"]

    bf = ml_dtypes.bfloat16
    # [S, E] -> [P, NSQB, NE, SQB]: xT[p, t, n, s] = x[t*SQB+s, n*P+p]
    xTq = np.ascontiguousarray(
        x.reshape(NSQB, SQB, NE, P).transpose(3, 0, 2, 1).astype(bf)
    )
    WqT = np.ascontiguousarray(Wq.T)
    WkT = np.ascontiguousarray(Wk.T)
    WvT = np.ascontiguousarray(Wv.T)
    WoT = np.ascontiguousarray(Wo.T)

    in_maps = []
    for i in range(NCORES):
        sl = slice(i * CSL, (i + 1) * CSL)

        def wslice(WT):
            # [E, CSL] slice -> [P, NE, CSL] partition-major
            return np.ascontiguousarray(
                WT[:, sl].reshape(NE, P, CSL).transpose(1, 0, 2).astype(bf)
            )

        in_maps.append({
            "xT": xTq,
            "ident": np.eye(P, dtype=np.float32).astype(bf),
            "wqT": wslice(WqT),
            "wkT": wslice(WkT),
            "wvT": wslice(WvT),
            "woT": np.ascontiguousarray(WoT[sl, :].astype(bf)),
        })

    last_results = run_bass_kernel_spmd(
        nc, in_maps, core_ids=list(range(NCORES)), trace=TRACE
    )
    out = np.zeros((S, E), dtype=np.float32)
    for r in last_results.results:
        out += r["y"].astype(np.float32).reshape(S, E)
    return out


# revision 20
# speedup vs baseline: 1.2592x; 1.0250x over previous
"""Multi-head attention TRN2 Bass kernel, head-sharded across 8 NeuronCores.

Problem: S=2048, E=1024, H=16 heads, dk=dv=64, fp32.
    Q = x @ Wq.T ; K = x @ Wk.T ; V = x @ Wv.T   (per-head slices)
    A_h = softmax(Q_h K_h^T / 8) V_h
    out = concat_h(A_h) @ Wo.T
Sharding: tensor-parallel over heads. Core i owns heads (2i, 2i+1); the 8
partial [2048,1024] outputs are summed on the host.

v7 layout (per-core):
  * Scores run as row-tiled K=64 matmul pairs (tile_position (0,0)/(64,0)):
    both heads' score chunks execute CONCURRENTLY in the PE array into
    adjacent PSUM banks, halving score time and killing the zero-padded
    kpad scheme of v4-v6.
  * One exp ACTIVATE per chunk covers both heads ([128, 2x512] psum).
  * Scores run up to a block ahead of their AV consumption (deep es ring):
    while block b's AV/normalize/outproj drains, block b+1's scores keep
    the scalar engine (the kernel bottleneck at ~74us of exp) saturated.
  * Few, large HWDGE input transfers; first x quarter split across both
    queues. y DMAs ride the sync queue only (a dma_start costs ~0.6us of
    issuing-engine time; the scalar engine must spend it on exp instead).
  * Warm-up matmuls at t~0 open the HAM clock gate before real work.
  * PSUM: scores 2x[128,2,512] + AV accum 2x[128,512] + kq 1 + v 1 = 8.
All matmul operands bf16 (fp32 PSUM accumulation). AV rides the ones-column
trick for softmax denominators (head B offset so both normalize multiplies
stay in-lane).
"""

from collections import deque

import numpy as np
import ml_dtypes

import concourse.mybir as mybir
import concourse.tile as tile
from concourse import bacc
from concourse.bass_utils import run_bass_kernel_spmd

S, E, H, DK, DV = 2048, 1024, 16, 64, 64
NCORES = 8
HPC = H // NCORES          # heads per core = 2
CSL = HPC * DV             # concat-dim columns per core = 128
P = 128
NE = E // P                # 8 contraction chunks for projections
SQB = 512                  # sequence block (PSUM-bank-limited matmul width)
NSQB = S // SQB            # 4
NCH = S // P               # 16 sk chunks of 128
F32 = mybir.dt.float32
BF16 = mybir.dt.bfloat16
SCALE = 1.0 / np.sqrt(DK).astype(np.float32)  # 1/8

EXP = mybir.ActivationFunctionType.Exp
MULT = mybir.AluOpType.mult

_cache = {}
last_results = None  # BassKernelResults of the most recent run (for test.py)
TRACE = False


def _build_nc():
    nc = bacc.Bacc("TRN2", target_bir_lowering=False, debug=False)

    xT = nc.dram_tensor("xT", [P, NSQB, NE, SQB], BF16, kind="ExternalInput")
    wqT = nc.dram_tensor("wqT", [P, NE, CSL], BF16, kind="ExternalInput")
    wkT = nc.dram_tensor("wkT", [P, NE, CSL], BF16, kind="ExternalInput")
    wvT = nc.dram_tensor("wvT", [P, NE, CSL], BF16, kind="ExternalInput")
    woT = nc.dram_tensor("woT", [CSL, E], BF16, kind="ExternalInput")
    ident = nc.dram_tensor("ident", [P, P], BF16, kind="ExternalInput")
    y = nc.dram_tensor("y", [NCH, P, E], BF16, kind="ExternalOutput")

    xT_r = xT.ap()
    w_r = {"q": wqT.ap(), "k": wkT.ap(), "v": wvT.ap()}
    y_ap = y.ap()

    with tile.TileContext(nc) as tc:
        with tc.tile_pool(name="persist", bufs=1) as persist, \
             tc.tile_pool(name="xw", bufs=1) as xw:
            qt = persist.tile([P, S], BF16)   # QT, heads on partitions 0-63/64-127
            kt = persist.tile([P, S], BF16)   # KT, same head split
            vt = persist.tile([P, S], BF16, name="vt", tag="vt")
            # head A V-block: [V(64) | ones(2)]; head B: [32 zeros | ones(2) |
            # 30 zeros | V]: its attention output lands on partitions 64-127
            # and its denominators on 32-33 (32-aligned for custom-DVE reads)
            vaug0 = persist.tile([P, NCH, DV + 2], BF16, name="vaug0", tag="vaug0")
            vaug1 = persist.tile([P, NCH, P], BF16, name="vaug1", tag="vaug1")
            wosb = persist.tile([P, E], BF16)
            idsb = persist.tile([P, P], BF16, name="idsb", tag="idsb")
            warmsb = persist.tile([P, SQB], BF16, name="warmsb", tag="warmsb")

            # ---- DMA issue first: 2 HWDGE queues, big transfers ----
            wsb = {}
            for m in ("k", "q", "v"):
                wsb[m] = xw.tile([P, NE, CSL], BF16, name=f"w{m}sb", tag=f"w{m}")
            xq = [
                xw.tile([P, NE, SQB], BF16, name=f"xq{t}", tag=f"xq{t}")
                for t in range(NSQB)
            ]
            # stripe each x quarter across both queues so quarters complete
            # incrementally (projections chase half-quarters via subtile deps)
            nc.sync.dma_start(xq[0][:, 0:4, :], xT_r[:, 0, 0:4, :])
            nc.scalar.dma_start(wsb["k"][:], w_r["k"][:])
            nc.scalar.dma_start(wsb["q"][:], w_r["q"][:])
            nc.scalar.dma_start(xq[0][:, 4:8, :], xT_r[:, 0, 4:8, :])
            nc.sync.dma_start(wsb["v"][:], w_r["v"][:])
            nc.sync.dma_start(idsb[:], ident.ap())
            nc.sync.dma_start(xq[1][:, 0:4, :], xT_r[:, 1, 0:4, :])
            nc.scalar.dma_start(xq[1][:, 4:8, :], xT_r[:, 1, 4:8, :])
            nc.sync.dma_start(xq[2][:, 0:4, :], xT_r[:, 2, 0:4, :])
            nc.scalar.dma_start(xq[2][:, 4:8, :], xT_r[:, 2, 4:8, :])
            nc.sync.dma_start(xq[3][:, 0:4, :], xT_r[:, 3, 0:4, :])
            nc.scalar.dma_start(xq[3][:, 4:8, :], xT_r[:, 3, 4:8, :])
            nc.sync.dma_start(wosb[:], woT.ap())

            # warm the ACT exp table set right after the DMA dispatches
            warm = persist.tile([1, 16], F32, name="warm", tag="warm")
            warm2 = persist.tile([1, 16], F32, name="warm2", tag="warm2")
            nc.gpsimd.memset(warmsb[:], 0.25)
            nc.gpsimd.memset(warm[:], 0.0)
            nc.scalar.activation(warm2[:], warm[:], EXP)

            nc.gpsimd.memset(vaug0[:, :, DV : DV + 2], 1.0)
            nc.gpsimd.memset(vaug1[:, :, 0:32], 0.0)
            nc.gpsimd.memset(vaug1[:, :, 32:34], 1.0)
            nc.gpsimd.memset(vaug1[:, :, 34:DV], 0.0)

            with tc.tile_pool(name="ps", bufs=1, space="PSUM") as ps, \
                 tc.tile_pool(name="est", bufs=18) as est_pool, \
                 tc.tile_pool(name="a1t", bufs=2) as a1t_pool, \
                 tc.tile_pool(name="small", bufs=2) as small, \
                 tc.tile_pool(name="outp", bufs=4) as outp:

                # ---- PE warm-up: junk matmuls from t~0 so the HAM
                # un-throttles before the first projection ----
                wps = ps.tile([P, 2, SQB], F32, name="wps", tag="sc", bufs=2)
                for i in range(12):
                    nc.tensor.matmul(
                        wps[:, 0, :], lhsT=warmsb[:, 0:P], rhs=warmsb[:],
                        start=True, stop=True,
                    )

                # ---- projections (per quarter) ----
                def proj(t, which):
                    sl = slice(t * SQB, (t + 1) * SQB)
                    tag = "v" if which == "v" else "kq"
                    pp = ps.tile(
                        [P, SQB], F32, name=f"p{which}{t}", tag=tag, bufs=1
                    )
                    for n in range(NE):
                        nc.tensor.matmul(
                            pp[:], lhsT=wsb[which][:, n, :], rhs=xq[t][:, n, :],
                            start=(n == 0), stop=(n == NE - 1),
                        )
                    dst = {"q": qt, "k": kt, "v": vt}[which]
                    nc.vector.tensor_copy(dst[:, sl], pp[:])

                def transp_v(t):
                    vp = ps.tile(
                        [P, 4, P], BF16, name=f"vp{t}", tag="kq", bufs=1
                    )
                    for j in range(4):
                        c = 4 * t + j
                        nc.tensor.transpose(
                            vp[:, j, :], vt[:, c * P : (c + 1) * P], idsb[:]
                        )
                        nc.vector.tensor_copy(vaug0[:, c, 0:DV], vp[:, j, 0:DV])
                        nc.vector.tensor_copy(vaug1[:, c, DV:P], vp[:, j, DV:P])

                # ---- attention chunk granules: scores for chunk c are a
                # row-tiled concurrent pair (head A rows 0-63, head B rows
                # 64-127), one exp ACTIVATE covers both heads ----
                sq = {b: deque() for b in range(NSQB)}  # (c, es) awaiting AV
                nxt = {b: 0 for b in range(NSQB)}       # next score chunk
                at_tiles = {}

                def s_chunk(b):
                    c = nxt[b]
                    nxt[b] += 1
                    bsl = slice(b * SQB, (b + 1) * SQB)
                    csl = slice(c * P, (c + 1) * P)
                    pss = ps.tile(
                        [P, 2, SQB], F32, name=f"ss{b}_{c}", tag="sc", bufs=2
                    )
                    nc.tensor.matmul(
                        pss[:, 0, :], lhsT=kt[0:DK, csl], rhs=qt[0:DK, bsl],
                        start=True, stop=True, tile_position=(0, 0),
                    )
                    nc.tensor.matmul(
                        pss[:, 1, :], lhsT=kt[DK:P, csl], rhs=qt[DK:P, bsl],
                        start=True, stop=True, tile_position=(64, 0),
                    )
                    es = est_pool.tile(
                        [P, 2, SQB], BF16, name=f"es{b}_{c}", tag="est"
                    )
                    nc.scalar.activation(es[:], pss[:], EXP, scale=float(SCALE))
                    sq[b].append((c, es))

                def av_chunk(b):
                    c, es = sq[b].popleft()
                    at_ps = at_tiles[b]
                    nc.tensor.matmul(
                        at_ps[0][0 : DV + 2, :],
                        lhsT=vaug0[:, c, :], rhs=es[:, 0, :],
                        start=(c == 0), stop=(c == NCH - 1),
                    )
                    nc.tensor.matmul(
                        at_ps[1][:],
                        lhsT=vaug1[:, c, :], rhs=es[:, 1, :],
                        start=(c == 0), stop=(c == NCH - 1),
                    )

                def norm_head(b, a1t, h, last):
                    at_ps = at_tiles[b]
                    src = at_ps[h][DV : DV + 1, :] if h == 0 else at_ps[h][32:33, :]
                    rs = small.tile([1, SQB], F32, tag=f"rs{h}")
                    if last:
                        nc.scalar.copy(rs[:], src)  # ACT is idle at the end
                    else:
                        nc.vector.tensor_copy(rs[:], src)
                    rsr = small.tile([1, SQB], F32, tag=f"rsr{h}")
                    nc.vector.reciprocal_approx_fast(rsr[:], rs[:])
                    bc = small.tile([P, SQB], F32, tag=f"bc{h}")
                    nc.gpsimd.partition_broadcast(bc[:], rsr[:])
                    rows = slice(0, DV) if h == 0 else slice(DV, P)
                    nc.vector.tensor_tensor(
                        a1t[rows, :], at_ps[h][rows, :], bc[rows, :], MULT
                    )

                def norm_block(b, last=False):
                    a1t = a1t_pool.tile([P, SQB], BF16, name=f"a1t{b}", tag="a1t")
                    if last:
                        # split the final chunk's AV by head: head-B's matmul
                        # runs on the PE while head-A's normalize chain
                        # (DVE/gpsimd) is already going
                        c, es = sq[b].popleft()
                        at_ps = at_tiles[b]
                        nc.tensor.matmul(
                            at_ps[0][0 : DV + 2, :],
                            lhsT=vaug0[:, c, :], rhs=es[:, 0, :],
                            start=False, stop=True,
                        )
                        norm_head(b, a1t, 0, last)
                        nc.tensor.matmul(
                            at_ps[1][:],
                            lhsT=vaug1[:, c, :], rhs=es[:, 1, :],
                            start=False, stop=True,
                        )
                        norm_head(b, a1t, 1, last)
                    else:
                        norm_head(b, a1t, 0, last)
                        norm_head(b, a1t, 1, last)
                    return a1t

                def outproj_j(b, a1t, j, last):
                    osb = outp.tile([P, E], BF16, tag="osb")
                    for e2 in range(E // SQB):
                        esl = slice(e2 * SQB, (e2 + 1) * SQB)
                        if last and e2 == 0:
                            ops = ps.tile(
                                [P, SQB], F32, name=f"op{b}_{j}_{e2}",
                                tag="sc", bufs=2,
                            )
                        else:
                            ops = ps.tile(
                                [P, SQB], F32, name=f"op{b}_{j}_{e2}",
                                tag=("kq" if e2 == 0 else "v"), bufs=1,
                            )
                        nc.tensor.matmul(
                            ops[:],
                            lhsT=a1t[:, j * P : (j + 1) * P],
                            rhs=wosb[:, esl],
                            start=True, stop=True,
                        )
                        if last and e2 == 0:
                            nc.scalar.copy(osb[:, esl], ops[:])
                        else:
                            nc.vector.tensor_copy(osb[:, esl], ops[:])
                    nc.sync.dma_start(y_ap[NSQB * b + j, :, :], osb[:])

                # ---- phase 1: quarters, block-0 attention interleaved,
                # block-1 scores as ACT filler once Q1 exists ----
                at_tiles[0] = [
                    ps.tile([P, SQB], F32, name=f"at0_{h}", tag="av", bufs=2)
                    for h in range(HPC)
                ]
                for t in range(NSQB):
                    proj(t, "k")
                    if t == 0:
                        proj(0, "q")
                    if t == 1:
                        proj(1, "q")
                    s_chunk(0)
                    if t > 0:
                        av_chunk(0)
                    s_chunk(0)
                    if t > 0:
                        av_chunk(0)
                    proj(t, "v")
                    transp_v(t)
                    s_chunk(0)
                    av_chunk(0)
                    s_chunk(0)
                    av_chunk(0)
                    # block-1 score-ahead (ACT filler): 4 at t=1, 4 at t=2,
                    # 2 at t=3 -> block 1 enters phase 2 with 10 queued
                    if t == 1 or t == 2:
                        for _ in range(4):
                            s_chunk(1)
                    elif t == 3:
                        for _ in range(2):
                            s_chunk(1)
                # b0 avs lag 2 behind; drained at phase-2 entry

                # ---- phase 2: blocks 1..3 with next-block score-ahead ----
                PRE = 10  # chunks pre-queued for the next block
                for b in range(1, NSQB):
                    while sq[b - 1]:
                        av_chunk(b - 1)
                    a1t_prev = norm_block(b - 1)
                    # allocate AFTER norm_block so the psum-slot reuse sees
                    # the normalize reads of block b-1
                    at_tiles[b] = [
                        ps.tile(
                            [P, SQB], F32, name=f"at{b}_{h}", tag="av", bufs=2
                        )
                        for h in range(HPC)
                    ]
                    if b + 1 < NSQB:
                        proj(b + 1, "q")
                    opj = 0
                    nbeats = NCH if b < NSQB - 1 else NCH - 1
                    for i in range(nbeats):
                        if not sq[b]:
                            s_chunk(b)
                        av_chunk(b)
                        if i % 4 == 1 and opj < NSQB:
                            outproj_j(b - 1, a1t_prev, opj, last=False)
                            opj += 1
                        if nxt[b] < NCH:
                            s_chunk(b)
                        elif b + 1 < NSQB and nxt[b + 1] < PRE:
                            s_chunk(b + 1)
                # last block: final AV overlaps the head-A normalize chain
                a1t_last = norm_block(NSQB - 1, last=True)
                for j in range(NSQB):
                    outproj_j(NSQB - 1, a1t_last, j, last=True)

    nc.compile()
    return nc


def kernel(x, Wq, Wk, Wv, Wo):
    global last_results
    x = np.asarray(x, dtype=np.float32)
    Wq = np.asarray(Wq, dtype=np.float32)
    Wk = np.asarray(Wk, dtype=np.float32)
    Wv = np.asarray(Wv, dtype=np.float32)
    Wo = np.asarray(Wo, dtype=np.float32)

    if "nc" not in _cache:
        _cache["nc"] = _build_nc()
    nc = _cache["nc"]

    bf = ml_dtypes.bfloat16
    # [S, E] -> [P, NSQB, NE, SQB]: xT[p, t, n, s] = x[t*SQB+s, n*P+p]
    xTq = np.ascontiguousarray(
        x.reshape(NSQB, SQB, NE, P).transpose(3, 0, 2, 1).astype(bf)
    )
    WqT = np.ascontiguousarray(Wq.T)
    WkT = np.ascontiguousarray(Wk.T)
    WvT = np.ascontiguousarray(Wv.T)
    WoT = np.ascontiguousarray(Wo.T)

    in_maps = []
    for i in range(NCORES):
        sl = slice(i * CSL, (i + 1) * CSL)

        def wslice(WT):
            # [E, CSL] slice -> [P, NE, CSL] partition-major
            return np.ascontiguousarray(
                WT[:, sl].reshape(NE, P, CSL).transpose(1, 0, 2).astype(bf)
            )

        in_maps.append({
            "xT": xTq,
            "ident": np.eye(P, dtype=np.float32).astype(bf),
            "wqT": wslice(WqT),
            "wkT": wslice(WkT),
            "wvT": wslice(WvT),
            "woT": np.ascontiguousarray(WoT[sl, :].astype(bf)),
        })

    last_results = run_bass_kernel_spmd(
        nc, in_maps, core_ids=list(range(NCORES)), trace=TRACE
    )
    out = np.zeros((S, E), dtype=np.float32)
    for r in last_results.results:
        out += r["y"].astype(np.float32).reshape(S, E)
    return out
